# revision 21
# baseline (speedup 1.0000x reference)
"""BiAttention Trainium2 kernel (8 NeuronCores, batch-parallel).

Problem (per batch element b, 8 of them -> one per core):
    A_proj = A @ W_A + b_A            [2048, 64]
    B_proj = B @ W_B + b_B            [2048, 64]
    S      = A_proj @ B_proj^T        [2048, 2048]
    A_star = softmax(S, axis=-1) @ B  [2048, 768]
    B_star = softmax(S, axis=0)^T @ A [2048, 768]

Key algebra used on-device (S is small: |S| < ~30, so exp(S) is safe in
f32/bf16 without max-subtraction):
    E = exp(S)
    A_star = diag(1/rowsum(E)) . (E @ B)
    B_star = diag(1/colsum(E)) . (E^T @ A)
rowsum/colsum are obtained for free by augmenting the moving operands
with a ones-column (E @ [B | 1] gives the row sums in the last column).

E is never materialized in full: score panels are recomputed per
512-wide output stripe (K=64 contraction - cheap) directly from the
projections, exp'd into bf16 packs, and immediately consumed as the
stationary operand of the big accumulation matmuls (which run at the
PE roofline: 769 cycles per j-pair, LDWEIGHTS hidden).

Design notes (each trace-verified on HW):
  - NO on-device transposes: the host passes A^T/B^T pre-arranged in
    the exact SBUF layout the projection matmuls consume, and ALL
    inputs host-cast to bf16 (plain parallel DMAs on the sync HWDGE +
    gpsimd SWDGE queues; ~13MB input resident by ~40us).  PE identity
    transposes cost ~160ns per [128,128] block (serial LDWEIGHTS);
    DMA xbar transposes serialize against all other DMA traffic.
  - projT rows 64:128 come from host-doubled weights (W2 = [W | W]):
    the projection matmul writes all 128 rows in one shot, enabling
    row-packed (tile_position (0,0)/(64,0)) K=64 score matmuls.
  - Warmup + woven filler matmuls (gated only on an SBUF memset,
    targeting transient spack psum tiles) keep the PE busy through the
    DMA-bound lead-in so the HAM clock gate opens at ~3.4us and never
    re-throttles (a >3.4us idle window halves the PE clock).
  - Lead-in streams the first stripe's ii=0 accumulation against the
    DMA arrival order; later rounds run dense from SBUF.
  - Packs for item n+1 are woven as one 2-pair run per tile (the
    second pair's LDWEIGHTS hides in the background weight buffer;
    longer runs stall on the 4-deep spack psum ring / exp latency).
  - Final tile accumulates its rowsum bank into a separate psum tile
    so the reciprocal + a third of the output drain overlap the last
    matmuls; the remaining output drains in chunks.
"""

import sys

if "/opt/trn_rl_repo" not in sys.path:
    sys.path.insert(0, "/opt/trn_rl_repo")

import numpy as np

import concourse.bass as bass
import concourse.mybir as mybir
import concourse.tile as tile
from concourse import bacc
from concourse.bass import ts
from concourse.bass_utils import run_bass_kernel_spmd
F32 = mybir.dt.float32
BF16 = mybir.dt.bfloat16
AF = mybir.ActivationFunctionType

L = 2048          # sequence length (both La and Lb)
D = 768           # model dim
H = 64            # projection dim
H2 = 2 * H        # doubled projection rows (host-doubled weights)
NT = L // 128     # 16 row/col tiles of 128
KD = D // 128     # 6 contraction tiles for the projections
NSUP = L // 512   # 4 supers (512-wide output stripes)
DP = D + 1        # moving operand width with the ones column
N_FILL = 16       # warmup filler matmuls (N=512): PE busy till ~11.4us

N_CORES = 8

_CACHE = {}


def _build():
    nc = bacc.Bacc("TRN2", target_bir_lowering=False, debug=False,
                   num_devices=N_CORES)
    A_d = nc.dram_tensor("A", [L, D], BF16, kind="ExternalInput").ap()
    B_d = nc.dram_tensor("B", [L, D], BF16, kind="ExternalInput").ap()
    # host-transposed inputs, pre-arranged in the mts SBUF layout:
    # XT[p, i*KD + k, q] = X[i*128 + q, k*128 + p]
    AT_d = nc.dram_tensor("AT", [128, NT * KD, 128], BF16,
                          kind="ExternalInput").ap()
    BT_d = nc.dram_tensor("BT", [128, NT * KD, 128], BF16,
                          kind="ExternalInput").ap()
    WA_d = nc.dram_tensor("W2_A", [D, H2], BF16, kind="ExternalInput").ap()
    WB_d = nc.dram_tensor("W2_B", [D, H2], BF16, kind="ExternalInput").ap()
    bA_d = nc.dram_tensor("b2_A", [H2, 1], F32, kind="ExternalInput").ap()
    bB_d = nc.dram_tensor("b2_B", [H2, 1], F32, kind="ExternalInput").ap()
    AS_d = nc.dram_tensor("A_star", [L, D], F32, kind="ExternalOutput").ap()
    BS_d = nc.dram_tensor("B_star", [L, D], F32, kind="ExternalOutput").ap()

    with tile.TileContext(nc) as tc:
        with (
            tc.tile_pool(name="mov", bufs=1) as pmov,
            tc.tile_pool(name="proj", bufs=1) as pproj,
            tc.tile_pool(name="pack", bufs=2) as ppack,
            tc.tile_pool(name="outp", bufs=4) as pout,
            tc.tile_pool(name="psum", bufs=2, space="PSUM") as pps,
        ):
            dram = {"A": A_d, "B": B_d}
            dramT = {"A": AT_d, "B": BT_d}
            stg = {"A": [], "B": []}   # persistent [128, 2, DP] units
            mts = {}
            projT = {}
            w_sb = {}
            b_sb = {}
            for side in ("A", "B"):
                for u in range(NT // 2):
                    stg[side].append(pmov.tile(
                        [128, 2, DP], BF16, tag=f"stg{side}{u}",
                        name=f"stg{side}{u}"))
                mts[side] = pmov.tile([128, NT * KD, 128], BF16,
                                      tag=f"t{side}", name=f"{side}_T")
                # all 128 rows written by the proj activation (host
                # doubles W/b), so K=64 score matmuls can be row-packed
                # two-at-a-time with tile_position (0,0)/(64,0)
                projT[side] = pproj.tile([128, L], BF16, tag=f"p{side}",
                                         name=f"{side}_projT")

            def kick_load(side, u):
                # one casting DMA per 2-tile unit: f32 DRAM -> bf16 SBUF
                # (SWDGE), writing the data columns of the persistent
                # staging tile (col D stays the memset ones column)
                nc.gpsimd.dma_start(
                    out=stg[side][u][:, :, 0:D],
                    in_=dram[side][u * 256:(u + 1) * 256, :].rearrange(
                        "(t p) d -> p t d", p=128
                    ),
                )

            def kick_loadT(side, t0, nt):
                # plain bf16 load of the host-transposed input straight
                # into mts (s-tiles t0 .. t0+nt-1, all KD d-blocks) on
                # the sync HWDGE queue, parallel to the gpsimd stream
                # (keeping BT/AT interleaved on ONE queue staggers their
                # arrivals gently; splitting across sync+scalar was
                # measured to open a >3.4us PE hole -> HAM re-throttle)
                nc.sync.dma_start(
                    out=mts[side][:, t0 * KD:(t0 + nt) * KD, :],
                    in_=dramT[side][:, t0 * KD:(t0 + nt) * KD, :],
                )

            def kick_weights(side, W_dram):
                wb = pmov.tile([128, KD, H2], BF16, tag=f"w{side}",
                               name=f"w{side}b")
                nc.gpsimd.dma_start(
                    out=wb, in_=W_dram.rearrange("(k p) h -> p k h", p=128)
                )
                w_sb[side] = wb

            def proj_chunk(side, c, parts=1):
                # projT[:, 512c:512(c+1)] = sum_d W2[d,:] M^T[d, s-chunk]
                # (cols 0:64 and 64:128 of W2 are identical, so rows
                # 64:128 of projT duplicate rows 0:64); parts=2 splits
                # into 256-col halves so the first chunk can start on a
                # half-arrived transposed load
                mtv = mts[side].rearrange("p (i j) q -> p i j q", j=KD)
                w = 4 // parts
                for h in range(parts):
                    ps = pps.tile([128, 512 // parts], F32, tag="spack",
                                  bufs=4, name=f"psproj{side}{c}{h}")
                    for k in range(KD):
                        nc.tensor.matmul(
                            ps,
                            w_sb[side][:, k, :],
                            mtv[:, 4 * c + h * w:4 * c + (h + 1) * w, k, :],
                            start=(k == 0), stop=(k == KD - 1),
                        )
                    nc.scalar.activation(
                        out=projT[side][:, c * 512 + h * (512 // parts):
                                        c * 512 + (h + 1) * (512 // parts)],
                        in_=ps,
                        func=AF.Identity, bias=b_sb[side], scale=1.0,
                    )

            # ---- kicks: biases (scalar HWDGE), inputs+weights (gpsimd
            # SWDGE, ordered by first consumption) ----
            for side, b_dram in (("A", bA_d), ("B", bB_d)):
                bt = pmov.tile([H2, 1], F32, tag=f"b{side}", name=f"b{side}sb")
                nc.scalar.dma_start(out=bt, in_=b_dram)
                b_sb[side] = bt
            # sync HWDGE stream: transposed chunks (proj operands)
            kick_loadT("B", 0, 2)       # BT chunk 0 first half
            kick_loadT("B", 2, 2)       # BT chunk 0 second half
            kick_loadT("A", 0, 4)       # AT chunk 0
            kick_loadT("B", 4, 4)       # BT chunk 1
            kick_loadT("A", 4, 4)       # AT chunk 1
            kick_loadT("B", 8, 4)       # BT chunk 2
            kick_loadT("B", 12, 4)      # BT chunk 3
            kick_loadT("A", 8, 4)       # AT chunk 2
            kick_loadT("A", 12, 4)      # AT chunk 3
            # gpsimd SWDGE stream (parallel): weights + natural units
            kick_weights("B", WB_d)
            kick_weights("A", WA_d)
            for u in range(8):
                kick_load("B", u)
            for u in range(8):
                kick_load("A", u)

            # ones columns (vector; disjoint from the load columns)
            for side in ("A", "B"):
                for u in range(NT // 2):
                    nc.vector.memset(stg[side][u][:, :, D:DP], 1.0)

            # HAM warmup + fillers: matmuls that depend only on an SBUF
            # memset keep the PE busy from ~6.3us (end of framework
            # preamble; opens the HAM clock gate) until the first
            # projection data lands (~12.5us)
            wmov = pmov.tile([128, 512], BF16, tag="warmmv", name="warmmv")
            nc.vector.memset(wmov, 0.125)
            wps = pps.tile([128, 1024], F32, tag="accum", name="warmps")

            def fill(n):
                # dependency-free matmuls into the warmup psum slot:
                # absorb expected DMA-arrival waits so the PE never sits
                # idle >3.4us (which would drop the HAM clock to 1.2GHz).
                # Only legal while the accum pool's slot 0 still belongs
                # to wps (i.e. before pa01 is created).
                for _ in range(n):
                    nc.tensor.matmul(wps[:, 0:512], wmov[:, 0:128], wmov,
                                     start=True, stop=True)

            fill(N_FILL)

            # ---- main: per 512-wide output stripe ----
            # dirn "A": produce A_star rows; panels are E'[t, s-stripe]
            #   (lhsT = B_projT tiles, rhs = A_projT stripe), moving = B stg
            # dirn "B": produce B_star rows; panels are E[s, t-stripe]
            #   (lhsT = A_projT tiles, rhs = B_projT stripe), moving = A stg
            work = [("A", u) for u in range(NSUP)] + \
                   [("B", u) for u in range(NSUP)]
            spec = {
                "A": (projT["B"], projT["A"], stg["B"], AS_d),
                "B": (projT["A"], projT["B"], stg["A"], BS_d),
            }
            packs = {}

            def pack_pair(w, jp):
                dirn, u = w
                pT_l, pT_r, _, _ = spec[dirn]
                pkt = ppack.tile([128, 1024], BF16, tag="pack", bufs=26,
                                 name=f"pk{dirn}{u}{jp}")
                for h2 in range(2):
                    # row-packed pair: K=64 matmuls in rows 0:64 / 64:128,
                    # each into its own 1-bank psum with its own exp so
                    # the ring recycles per-half
                    j = jp * 2 + h2
                    base = h2 * 64
                    ps = pps.tile([128, 512], F32, tag="spack", bufs=4,
                                  name=f"pss{dirn}{u}{jp}h{h2}")
                    nc.tensor.matmul(
                        ps,
                        pT_l[base:base + H, ts(j, 128)],
                        pT_r[base:base + H, ts(u, 512)],
                        start=True, stop=True,
                        tile_position=(base, 0),
                    )
                    nc.scalar.activation(out=pkt[:, ts(h2, 512)], in_=ps,
                                         func=AF.Exp)
                packs.setdefault(w, []).append(pkt)

            def accum_jpair(w, pa, jp, ii=0):
                dirn, u = w
                _, _, mv, _ = spec[dirn]
                pks = packs[w]
                for j in (2 * jp, 2 * jp + 1):
                    lhs = pks[j // 2][:, (j % 2) * 512 + ii * 128:
                                      (j % 2) * 512 + ii * 128 + 128]
                    mvt = mv[j // 2]
                    # short mm first: the trailing 512-col mm covers the
                    # next pair's LDWEIGHTS pull-ahead window
                    nc.tensor.matmul(
                        pa[:, 512:DP], lhs, mvt[:, j % 2, 512:DP],
                        start=(j == 0), stop=(j == NT - 1),
                    )
                    nc.tensor.matmul(
                        pa[:, 0:512], lhs, mvt[:, j % 2, 0:512],
                        start=(j == 0), stop=(j == NT - 1),
                    )

            def accum_tile_splitbank(w, paS, paL, ii):
                # last tile only: run all 16 short (cols 512:769, incl.
                # rowsum) matmuls into a SEPARATE psum tile first (so
                # its completion is tracked independently), then the 16
                # long ones.  The rowsum side completes ~3.5us before
                # the tile does; its reciprocal + the cols-512:768
                # output chunk drain concurrently with the long matmuls.
                dirn, u = w
                _, _, mv, _ = spec[dirn]
                pks = packs[w]
                for part in (1, 0):
                    for j in range(NT):
                        lhs = pks[j // 2][:, (j % 2) * 512 + ii * 128:
                                          (j % 2) * 512 + ii * 128 + 128]
                        mvt = mv[j // 2]
                        if part:
                            nc.tensor.matmul(
                                paS[:, 0:257], lhs, mvt[:, j % 2, 512:DP],
                                start=(j == 0), stop=(j == NT - 1),
                            )
                        else:
                            nc.tensor.matmul(
                                paL[:, 0:512], lhs, mvt[:, j % 2, 0:512],
                                start=(j == 0), stop=(j == NT - 1),
                            )

            def finish_tile(w, ii, pa, nchunk=1):
                dirn, u = w
                _, _, _, out_d = spec[dirn]
                rinv = pout.tile([128, 1], F32, tag="rinv",
                                 name=f"ri{dirn}{u}{ii}")
                nc.vector.reciprocal(out=rinv, in_=pa[:, D:DP])
                ot = pout.tile([128, D], F32, tag="ot",
                               name=f"ot{dirn}{u}{ii}")
                if nchunk == 1:
                    nc.vector.tensor_scalar_mul(ot, pa[:, 0:D], rinv)
                    nc.sync.dma_start(out=out_d[ts(u * 4 + ii, 128), :],
                                      in_=ot)
                else:
                    # chunked drain for the very last tile: pipeline the
                    # mul and output DMA to shorten the serial tail
                    cw = D // nchunk
                    for c in range(nchunk):
                        nc.vector.tensor_scalar_mul(
                            ot[:, ts(c, cw)], pa[:, ts(c, cw)], rinv)
                        nc.sync.dma_start(
                            out=out_d[ts(u * 4 + ii, 128), ts(c, cw)],
                            in_=ot[:, ts(c, cw)])

            # ---- lead-in: stream direction-A stripe 0 (ii=0 only)
            # against the DMA arrival order, with filler blocks sized to
            # the expected arrival waits (the lead-in is DMA-bound, so
            # fillers absorb idle that would otherwise trip the HAM
            # re-throttle); ii=1..3 then run dense from SBUF ----
            w0, w1, w2 = work[0], work[1], work[2]
            proj_chunk("B", 0, parts=2)
            fill(10)
            proj_chunk("A", 0)
            pack_pair(w0, 0)
            pack_pair(w0, 1)
            pa00 = pps.tile([128, 1024], F32, tag="accum", name="paA00")
            fill(4)
            accum_jpair(w0, pa00, 0, 0)
            accum_jpair(w0, pa00, 1, 0)
            fill(6)
            proj_chunk("B", 1)
            pack_pair(w0, 2)
            pack_pair(w0, 3)
            accum_jpair(w0, pa00, 2, 0)
            accum_jpair(w0, pa00, 3, 0)
            fill(6)
            proj_chunk("A", 1)
            pack_pair(w1, 0)
            pack_pair(w1, 1)
            pack_pair(w1, 2)
            pack_pair(w1, 3)
            fill(4)
            proj_chunk("B", 2)
            pack_pair(w0, 4)
            pack_pair(w0, 5)
            pack_pair(w1, 4)
            accum_jpair(w0, pa00, 4, 0)
            pack_pair(w1, 5)
            accum_jpair(w0, pa00, 5, 0)
            fill(4)
            proj_chunk("B", 3)
            pack_pair(w0, 6)
            pack_pair(w0, 7)
            pack_pair(w1, 6)
            accum_jpair(w0, pa00, 6, 0)
            pack_pair(w1, 7)
            accum_jpair(w0, pa00, 7, 0)
            finish_tile(w0, 0, pa00)

            # ii=1 / ii=2: dense accums from SBUF (covers the tail of
            # the transposed-chunk loads); projT_A stripe 2 woven in
            pa01 = pps.tile([128, 1024], F32, tag="accum", name="paA01")
            for jp in range(8):
                accum_jpair(w0, pa01, jp, 1)
            finish_tile(w0, 1, pa01)
            pa = pps.tile([128, 1024], F32, tag="accum", name="paA02")
            for jp in range(6):
                accum_jpair(w0, pa, jp, 2)
            proj_chunk("A", 2)
            accum_jpair(w0, pa, 6, 2)
            accum_jpair(w0, pa, 7, 2)
            finish_tile(w0, 2, pa)

            # w0 ii=3 with the first w2 packs woven in
            pa = pps.tile([128, 1024], F32, tag="accum", name="paA03")
            accum_jpair(w0, pa, 0, 3)
            accum_jpair(w0, pa, 1, 3)
            accum_jpair(w0, pa, 2, 3)
            pack_pair(w2, 0)
            accum_jpair(w0, pa, 3, 3)
            accum_jpair(w0, pa, 4, 3)
            accum_jpair(w0, pa, 5, 3)
            pack_pair(w2, 1)
            accum_jpair(w0, pa, 6, 3)
            accum_jpair(w0, pa, 7, 3)
            finish_tile(w0, 3, pa)

            # ---- steady pipeline: work items 1..7 ----
            # remaining proj chunks are woven right before the packs
            # that need them; packs for item n+1 are woven 1-pair-at-a-
            # time into item n's accum stream (spaced so the spack psum
            # ring and the scalar exp queue never back up)
            pair_cursor = {w1: 8, w2: 2}
            pre_ops = {
                (1, 0, 2): lambda: proj_chunk("A", 3),
            }

            def weave_pack(wn, k=1):
                c = pair_cursor.setdefault(wn, 0)
                while c < NT // 2 and k > 0:
                    pack_pair(wn, c)
                    c += 1
                    k -= 1
                pair_cursor[wn] = c

            for idx in range(1, len(work)):
                w = work[idx]
                wn = work[idx + 1] if idx + 1 < len(work) else None
                last = idx == len(work) - 1
                for ii in range(4):
                    pa = pps.tile([128, 1024], F32, tag="accum",
                                  name=f"pa{w[0]}{w[1]}{ii}")
                    if last and ii == 3:
                        paS = pps.tile([128, 512], F32, tag="spack",
                                       bufs=4, name="paSlast")
                        accum_tile_splitbank(w, paS, pa, ii)
                        dirn, u = w
                        out_d = spec[dirn][3]
                        rinv = pout.tile([128, 1], F32, tag="rinv",
                                         name="rilast")
                        nc.vector.reciprocal(out=rinv, in_=paS[:, 256:257])
                        ot = pout.tile([128, D], F32, tag="ot",
                                       name="otlast")
                        # cols 512:768 drain during the long matmuls
                        nc.vector.tensor_scalar_mul(
                            ot[:, 512:D], paS[:, 0:256], rinv)
                        nc.sync.dma_start(
                            out=out_d[ts(u * 4 + ii, 128), 512:D],
                            in_=ot[:, 512:D])
                        for c in range(2):
                            nc.vector.tensor_scalar_mul(
                                ot[:, ts(c, 256)], pa[:, ts(c, 256)], rinv)
                            nc.sync.dma_start(
                                out=out_d[ts(u * 4 + ii, 128), ts(c, 256)],
                                in_=ot[:, ts(c, 256)])
                        continue
                    for jp in range(8):
                        accum_jpair(w, pa, jp, ii)
                        op = pre_ops.get((idx, ii, jp))
                        if op is not None:
                            op()
                        elif pair_cursor.setdefault(w, 0) < NT // 2:
                            # finish this item's own late packs first
                            # (consumed a few j-pairs later this tile)
                            weave_pack(w)
                        elif wn is not None and jp == 3:
                            # 2-pair run: the second pair's first
                            # LDWEIGHTS hides in the background weight
                            # buffer under the first pair's matmuls
                            weave_pack(wn, 2)
                    finish_tile(w, ii, pa)
                packs.pop(work[idx - 1], None)

    nc.compile()
    return nc


def _get_nc():
    if "nc" not in _CACHE:
        _CACHE["nc"] = _build()
    return _CACHE["nc"]


BF = __import__("ml_dtypes").bfloat16


def _prep_T(X):
    # XT[p, i*KD + k, q] = X[i*128 + q, k*128 + p]  ->  [128, 96, 128]
    return np.ascontiguousarray(
        X.reshape(NT, 128, KD, 128).transpose(3, 0, 2, 1).reshape(
            128, NT * KD, 128))


def _run(inputs, trace=False):
    nc = _get_nc()
    # host-side bf16 cast (numerically identical to the dge-cast the
    # device used to do): halves input HBM traffic
    A = np.ascontiguousarray(np.asarray(inputs["A"], dtype=np.float32)
                             .astype(BF))
    B = np.ascontiguousarray(np.asarray(inputs["B"], dtype=np.float32)
                             .astype(BF))
    W_A = np.asarray(inputs["W_A"], dtype=np.float32)
    W_B = np.asarray(inputs["W_B"], dtype=np.float32)
    b_A = np.asarray(inputs["b_A"], dtype=np.float32).reshape(H)
    b_B = np.asarray(inputs["b_B"], dtype=np.float32).reshape(H)
    # host-doubled weights/biases: proj matmul writes projT rows 0:64
    # and 64:128 in one shot (replaces the SBUF->SBUF dup DMA)
    W2_A = np.ascontiguousarray(np.concatenate([W_A, W_A], axis=1)
                                .astype(BF))
    W2_B = np.ascontiguousarray(np.concatenate([W_B, W_B], axis=1)
                                .astype(BF))
    b2_A = np.ascontiguousarray(np.concatenate([b_A, b_A]).reshape(H2, 1))
    b2_B = np.ascontiguousarray(np.concatenate([b_B, b_B]).reshape(H2, 1))
    in_maps = [
        {
            "A": A[c], "B": B[c],
            "AT": _prep_T(A[c]), "BT": _prep_T(B[c]),
            "W2_A": W2_A, "W2_B": W2_B,
            "b2_A": b2_A, "b2_B": b2_B,
        }
        for c in range(N_CORES)
    ]
    res = run_bass_kernel_spmd(nc, in_maps, list(range(N_CORES)), trace=trace)
    A_star = np.stack([res.results[c]["A_star"] for c in range(N_CORES)])
    B_star = np.stack([res.results[c]["B_star"] for c in range(N_CORES)])
    return A_star, B_star, res


def kernel(**inputs):
    A_star, B_star, _ = _run(inputs)
    return A_star, B_star
